# revision 39
# baseline (speedup 1.0000x reference)
"""Distributed attention kernel for 8 trn2 NeuronCores (v2).

Reference semantics (B=2, S=2048, D=2048, H=16, dh=128):
  q = x@W_q, k = x@W_k  (per-head split), v = x@W_v (full width)
  scores = q@k^T per head; (scores + triu(-1e9)) * 1/sqrt(dh); softmax
  out = (sum_h probs_h) @ v @ W_o        <- heads summed, v full width

Sharding: 2 groups of 4 cores (batch parallel); within a group, rank r
owns heads {4r..4r+3} (cols of W_q/W_k). The value path is refactored:
  U = x @ (W_v @ W_o)   so   out = P @ U.
Each core computes WVO = W_v @ W_o[:, own 512 cols] (no collective),
then U[:, own 512] = x_g @ WVO, AllGathered early (overlaps q/k
projections). P_local = sum of own 4 heads' probs; per causal slab s
(4 q-tiles), P is stored right-sized [512, (s+1)*512] and
ReduceScatter(add) gives each rank 128 q-rows. Y_s = P_own_s @ U is
computed right after each slab's RS -> tiny serial tail.

Precision: score path (x@Wq, x@Wk, q@k^T) in float32r (full PE rate);
softmax in f32; P/U/WVO in bf16 with f32 PSUM accumulation.
"""

import math

import numpy as np
import ml_dtypes

import concourse.bass as bass
import concourse.mybir as mybir
import concourse.tile as tile
from concourse import bacc
from concourse.bass_utils import run_bass_kernel_spmd
from concourse.masks import make_identity

F32 = mybir.dt.float32
F32R = mybir.dt.float32r
BF16 = mybir.dt.bfloat16

S = 2048
D = 2048
DH = 128
NT = S // 128  # 16 q/k tiles
SCALE = 1.0 / math.sqrt(DH)
GROUPS = [[0, 1, 2, 3], [4, 5, 6, 7]]
NEG = -1e9


def build():
    nc = bacc.Bacc("TRN2", target_bir_lowering=False, debug=False, num_devices=8)

    x = nc.declare_dram_parameter("x", [D, S], F32R, isOutput=False)  # x TRANSPOSED on host
    xbf = nc.declare_dram_parameter("xbf", [D, S], BF16, isOutput=False)
    wq = nc.declare_dram_parameter("wq", [D, 512], F32R, isOutput=False)
    wk = nc.declare_dram_parameter("wk", [D, 512], F32R, isOutput=False)
    # W_v^T tiled [p, d, et]: [p, d, et] = W_v[d, et*128+p]
    wvt = nc.declare_dram_parameter("wvt", [128, D, 16], BF16, isOutput=False)
    # W_o own cols tiled [p, et, n]: [p, et, n] = W_o[et*128+p, r*512+n]
    wo = nc.declare_dram_parameter("wo", [128, 16, 512], BF16, isOutput=False)
    out = nc.declare_dram_parameter("out", [512, D], F32, isOutput=True)

    u_loc = nc.dram_tensor("u_loc", [S, 512], BF16)
    u_ag_a = nc.dram_tensor("u_ag_a", [4, 1024, 512], BF16)
    u_ag_b = nc.dram_tensor("u_ag_b", [4, 1024, 512], BF16)
    p_s = [nc.dram_tensor(f"p{s}", [512, (s + 1) * 512], BF16) for s in range(4)]
    po_s = [nc.dram_tensor(f"po{s}", [128, (s + 1) * 512], BF16) for s in range(4)]

    with tile.TileContext(nc) as tc:
        with tc.tile_pool(name="const", bufs=1) as cst:
            ident = cst.tile([128, 128], F32)
            make_identity(nc, ident)
            ident_bf = cst.tile([128, 128], BF16)
            nc.vector.tensor_copy(out=ident_bf[:], in_=ident[:])
            # ---------------- Phase A pools: resident weights + x quarters ----------------
            _qkp_cm = tc.tile_pool(name="qk_pool", bufs=1)
            qkp = _qkp_cm.__enter__()
            qT = qkp.tile([128, 4, S], F32R)  # [dh-part, head, q]
            kT = qkp.tile([128, 4, S], F32R)
            u_keep = qkp.tile([128, 2, 512], BF16)  # withheld u_loc tail tiles (AG gates)
            _wres_cm = tc.tile_pool(name="wres", bufs=1)
            wres = _wres_cm.__enter__()
            wq_sb = wres.tile([128, NT, 512], F32R)
            wk_sb = wres.tile([128, NT, 512], F32R)
            for Dt in range(NT):
                nc.sync.dma_start(wq_sb[:, Dt, :], wq[Dt * 128 : (Dt + 1) * 128, :])
                nc.sync.dma_start(wk_sb[:, Dt, :], wk[Dt * 128 : (Dt + 1) * 128, :])
            _xtp_cm = tc.tile_pool(name="xt_pool", bufs=1)
            xtp = _xtp_cm.__enter__()

            _psall_cm = tc.tile_pool(name="ps_all", bufs=8, space="PSUM")
            ps_all = _psall_cm.__enter__()

            def emit_A_quarter(qt, xq):
                """q/k projections for q-cols [qt*512, (qt+1)*512)."""
                psums = [
                    ps_all.tile([128, 512], F32, tag="ps512", name=f"prj{qt}_{_j}")
                    for _j in range(8)
                ]
                for Dt in range(NT):
                    for j in range(8):
                        wsb = wq_sb if j < 4 else wk_sb
                        nc.tensor.matmul(
                            psums[j][:],
                            wsb[:, Dt, (j % 4) * 128 : (j % 4) * 128 + 128],
                            xq[:, Dt, :],
                            start=(Dt == 0),
                            stop=(Dt == NT - 1),
                        )
                for j in range(8):
                    dst = qT if j < 4 else kT
                    nc.vector.tensor_copy(
                        out=dst[:, j % 4, qt * 512 : (qt + 1) * 512],
                        in_=psums[j][:],
                    )

            xqs = []
            for qt in range(4):
                xq = xtp.tile([128, NT, 512], F32R, tag="xt", name=f"xq{qt}")
                for Dt in range(NT):
                    # scalar-engine DMA queue: doesn't block sync-queue loads
                    nc.scalar.dma_start(
                        xq[:, Dt, :],
                        x[Dt * 128 : (Dt + 1) * 128, qt * 512 : (qt + 1) * 512],
                    )
                emit_A_quarter(qt, xq)

            # ---------------- Phase 0: WVO = Wv@Wo[:, own], then U = x@WVO ----------------
            with (
                tc.tile_pool(name="wvp", bufs=1) as wvp,
                tc.tile_pool(name="stream0", bufs=2) as stp,
                tc.tile_pool(name="osb", bufs=1) as osb,
            ):
                ps0 = ps_all
                wo_sb = wvp.tile([128, 16, 512], BF16)
                wvo_sb = wvp.tile([128, 16, 512], BF16)
                for et in range(16):
                    nc.sync.dma_start(wo_sb[:, et, :], wo[:, et, :])
                for grp in range(2):
                    psums = [
                        ps0.tile([128, 512], F32, tag="ps512", name=f"wvo{grp}_{j}")
                        for j in range(8)
                    ]
                    for j in range(8):
                        dt = grp * 8 + j
                        wvt_t = stp.tile([128, 128, 16], BF16, tag="wvt")
                        nc.sync.dma_start(
                            wvt_t[:], wvt[:, dt * 128 : (dt + 1) * 128, :]
                        )
                        for et in range(16):
                            nc.tensor.matmul(
                                psums[j][:],
                                wvt_t[:, :, et],
                                wo_sb[:, et, :],
                                start=(et == 0),
                                stop=(et == 15),
                            )
                    for j in range(8):
                        nc.vector.tensor_copy(
                            out=wvo_sb[:, grp * 8 + j, :], in_=psums[j][:]
                        )
                # U = x @ WVO, computed in 2 k-halves
                for sh in range(2):
                    psums = [
                        ps0.tile([128, 512], F32, tag="ps512", name=f"u{sh}_{j}")
                        for j in range(8)
                    ]
                    for Dt in range(NT):
                        xb_t = stp.tile([128, 1024], BF16, tag="xb")
                        nc.sync.dma_start(
                            xb_t[:],
                            xbf[Dt * 128 : (Dt + 1) * 128, sh * 1024 : sh * 1024 + 1024],
                        )
                        for j in range(8):
                            nc.tensor.matmul(
                                psums[j][:],
                                xb_t[:, j * 128 : (j + 1) * 128],
                                wvo_sb[:, Dt, :],
                                start=(Dt == 0),
                                stop=(Dt == NT - 1),
                            )
                    for j in range(8):
                        kt = sh * 8 + j
                        if j == 7:  # withhold: written late in C to gate the AG
                            nc.vector.tensor_copy(out=u_keep[:, sh, :], in_=psums[j][:])
                        else:
                            u_c = osb.tile([128, 512], BF16, tag="uc")
                            nc.vector.tensor_copy(out=u_c[:], in_=psums[j][:])
                            nc.sync.dma_start(u_loc[kt * 128 : (kt + 1) * 128, :], u_c[:])

            _psall_cm.__exit__(None, None, None)
            _xtp_cm.__exit__(None, None, None)
            _wres_cm.__exit__(None, None, None)

            # ---------------- u_sb load (gpsimd queue; waits on AG, overlaps phase C) ----------------
            _up_cm = tc.tile_pool(name="u_pool", bufs=1)
            up = _up_cm.__enter__()
            u_sb = up.tile([128, NT, D], BF16)  # [k-part, kt, n]
            u_gate = up.tile([128, 2, 512], BF16)
            # mask variant m: [128, 512], 0 where col <= row + 128*m else -1e9
            masks = up.tile([128, 4, 512], BF16)
            for m in range(4):
                nc.gpsimd.memset(masks[:, m, :], 0.0)
                nc.gpsimd.affine_select(
                    out=masks[:, m, :],
                    in_=masks[:, m, :],
                    compare_op=mybir.AluOpType.is_ge,
                    fill=NEG,
                    base=128 * m,
                    pattern=[[-1, 512]],
                    channel_multiplier=1,
                )
            # per-slab P^T tiles: pt_s [k-part, kt, own-q 128]
            pts = [up.tile([128, 4 * (s + 1), 128], BF16, name=f"pt{s}") for s in range(4)]
            def emit_ag(sh):
                # data-chained gate: the copy below executes on the in-order
                # vector queue inside phase C, so the DMA (and therefore the
                # AllGather's input-readiness) cannot precede it
                nc.vector.tensor_copy(out=u_gate[:, sh, :], in_=u_keep[:, sh, :])
                kt_g = sh * 8 + 7
                nc.gpsimd.dma_start(
                    u_loc[kt_g * 128 : (kt_g + 1) * 128, :], u_gate[:, sh, :]
                )
                nc.gpsimd.collective_compute(
                    "AllGather",
                    mybir.AluOpType.bypass,
                    ins=[u_loc[sh * 1024 : (sh + 1) * 1024, :]],
                    outs=[(u_ag_a if sh == 0 else u_ag_b)[:]],
                    replica_groups=GROUPS,
                )

            def emit_u_loads():
                for rr in range(4):
                    nc.gpsimd.dma_start(
                        u_sb[:, :8, rr * 512 : (rr + 1) * 512],
                        u_ag_a[rr].rearrange("(t p) n -> p t n", p=128),
                    )
                    nc.gpsimd.dma_start(
                        u_sb[:, 8:, rr * 512 : (rr + 1) * 512],
                        u_ag_b[rr].rearrange("(t p) n -> p t n", p=128),
                    )

            # ---------------- Phase C: scores / softmax / P (pipelined), RS, Y ----------------
            with (
                tc.tile_pool(name="epool", bufs=3) as ep,
                tc.tile_pool(name="small", bufs=32) as smp,
                tc.tile_pool(name="dsm", bufs=12) as dsm,
                tc.tile_pool(name="psb", bufs=4) as psbp,
                tc.tile_pool(name="ysb", bufs=2) as ysbp,
                tc.tile_pool(name="c_ps", bufs=1, space="PSUM") as cps,
            ):
                i_order = [i for sl in (3, 2, 1, 0) for i in range(4 * sl, 4 * sl + 4)]

                def emit_S(i):
                    """Scores + softmax for q-tile i; returns (e_t, d_hs, kwc)."""
                    kwc = i // 4 + 1
                    kw = kwc * 512
                    ntile = (kw + 1023) // 1024
                    dtid = (kw - 512) // 1024
                    doff = (kw - 512) % 1024
                    e_t = ep.tile([128, 4, 2048], BF16, tag="E", name=f"E{i}")
                    d_hs = []
                    for h in range(4):
                        s_tiles = [
                            cps.tile([128, 1024], F32, tag="S", bufs=3, name=f"sc{i}_{h}_{_j}")
                            for _j in range(ntile)
                        ]
                        for kc in range(kwc):
                            diag = kc == kwc - 1
                            tgt = s_tiles[kc // 2][:, (kc % 2) * 512 : (kc % 2) * 512 + 512]
                            if diag:
                                # preload causal mask into PSUM via identity matmul
                                nc.tensor.matmul(
                                    tgt,
                                    ident_bf[:],
                                    masks[:, i % 4, :],
                                    start=True,
                                    stop=False,
                                )
                            nc.tensor.matmul(
                                tgt,
                                qT[:, h, i * 128 : (i + 1) * 128],
                                kT[:, h, kc * 512 : (kc + 1) * 512],
                                start=not diag,
                                stop=True,
                                skip_group_check=True,
                            )
                        mx = None
                        for t in range(ntile):
                            w = min(kw - 1024 * t, 1024)
                            mxt = smp.tile([128, 1], F32, tag="mx")
                            nc.vector.reduce_max(
                                out=mxt[:],
                                in_=s_tiles[t][:, :w],
                                axis=mybir.AxisListType.X,
                            )
                            if mx is None:
                                mx = mxt
                            else:
                                mxn = smp.tile([128, 1], F32, tag="mx")
                                nc.vector.tensor_tensor(
                                    out=mxn[:],
                                    in0=mx[:],
                                    in1=mxt[:],
                                    op=mybir.AluOpType.max,
                                )
                                mx = mxn
                        nmS = smp.tile([128, 1], F32, tag="mx")
                        nc.vector.tensor_scalar_mul(nmS[:], mx[:], -SCALE)
                        rtot = None
                        for t in range(ntile):
                            w = min(kw - 1024 * t, 1024)
                            rc = smp.tile([128, 1], F32, tag="mx")
                            nc.scalar.activation(
                                out=e_t[:, h, 1024 * t : 1024 * t + w],
                                in_=s_tiles[t][:, :w],
                                func=mybir.ActivationFunctionType.Exp,
                                bias=nmS[:],
                                scale=SCALE,
                                accum_out=rc[:],
                            )
                            if rtot is None:
                                rtot = rc
                            else:
                                rn = smp.tile([128, 1], F32, tag="mx")
                                nc.vector.tensor_tensor(
                                    out=rn[:],
                                    in0=rtot[:],
                                    in1=rc[:],
                                    op=mybir.AluOpType.add,
                                )
                                rtot = rn
                        rinv = smp.tile([128, 1], F32, tag="mx")
                        nc.vector.reciprocal(out=rinv[:], in_=rtot[:])
                        d_h = dsm.tile([128, 128], BF16, tag="D")
                        nc.vector.tensor_scalar_mul(d_h[:], ident_bf[:], rinv[:])
                        d_hs.append(d_h)
                    return (e_t, d_hs, kwc)

                def emit_P(i, state):
                    e_t, d_hs, kwc = state
                    sl = i // 4
                    for kc in range(kwc):
                        p_t = cps.tile([128, 512], F32, tag="PY", bufs=2, name=f"pp{i}_{kc}")
                        for h in range(4):
                            nc.tensor.matmul(
                                p_t[:],
                                d_hs[h][:],
                                e_t[:, h, kc * 512 : (kc + 1) * 512],
                                start=(h == 0),
                                stop=(h == 3),
                            )
                        pc = psbp.tile([128, 512], BF16, tag="psb")
                        nc.scalar.copy(out=pc[:], in_=p_t[:])
                        nc.sync.dma_start(
                            p_s[sl][
                                (i % 4) * 128 : (i % 4) * 128 + 128,
                                kc * 512 : (kc + 1) * 512,
                            ],
                            pc[:],
                        )
                    if i % 4 == 3:  # slab complete on this rank -> RS
                        nc.gpsimd.collective_compute(
                            "ReduceScatter",
                            mybir.AluOpType.add,
                            ins=[p_s[sl][:]],
                            outs=[po_s[sl][:]],
                            replica_groups=GROUPS,
                        )
                        if sl == 3:
                            emit_ag(0)
                        elif sl == 2:
                            emit_ag(1)
                        elif sl == 1:
                            emit_u_loads()

                prev = None
                for i in i_order:
                    st = emit_S(i)
                    if prev is not None:
                        emit_P(*prev)
                    prev = (i, st)
                emit_P(*prev)
                # transposes after all P writes: no head-of-line blocking of p_s
                for sl in (3, 2, 1, 0):
                    for kt in range(4 * (sl + 1)):
                        nc.sync.dma_start_transpose(
                            pts[sl][:, kt, :],
                            po_s[sl][:, kt * 128 : (kt + 1) * 128],
                        )

                # ---------------- Y_s = P_own_s @ U ----------------
                for s in (3, 2, 1, 0):
                    nkt = 4 * (s + 1)
                    for nch in range(4):
                        yp = cps.tile([128, 512], F32, tag="PY", bufs=2, name=f"y{s}_{nch}")
                        for kt in range(nkt):
                            nc.tensor.matmul(
                                yp[:],
                                pts[s][:, kt, :],
                                u_sb[:, kt, nch * 512 : nch * 512 + 512],
                                start=(kt == 0),
                                stop=(kt == nkt - 1),
                            )
                        y_sb = ysbp.tile([128, 512], F32, tag="ysb")
                        nc.vector.tensor_copy(out=y_sb[:], in_=yp[:])
                        nc.sync.dma_start(
                            out[s * 128 : (s + 1) * 128, nch * 512 : nch * 512 + 512],
                            y_sb[:],
                        )

            _up_cm.__exit__(None, None, None)
            _qkp_cm.__exit__(None, None, None)

    nc.compile()
    return nc


_NC_CACHE = None


def _build_in_maps(x, W_q, W_k, W_v, W_o):
    xT = [np.ascontiguousarray(x[g].T) for g in range(2)]
    xT_bf = [t.astype(ml_dtypes.bfloat16) for t in xT]
    wvt = np.ascontiguousarray(
        W_v.T.reshape(16, 128, D).transpose(1, 2, 0)
    ).astype(ml_dtypes.bfloat16)
    in_maps = []
    for c in range(8):
        g, r = divmod(c, 4)
        wo_r = np.ascontiguousarray(
            W_o[:, 512 * r : 512 * (r + 1)].reshape(16, 128, 512).transpose(1, 0, 2)
        ).astype(ml_dtypes.bfloat16)
        in_maps.append(
            {
                "x": xT[g],
                "xbf": xT_bf[g],
                "wq": np.ascontiguousarray(W_q[:, 512 * r : 512 * (r + 1)]),
                "wk": np.ascontiguousarray(W_k[:, 512 * r : 512 * (r + 1)]),
                "wvt": wvt,
                "wo": wo_r,
            }
        )
    return in_maps


def kernel(x, W_q, W_k, W_v, W_o):
    global _NC_CACHE
    x = np.asarray(x, dtype=np.float32)
    W_q = np.asarray(W_q, dtype=np.float32)
    W_k = np.asarray(W_k, dtype=np.float32)
    W_v = np.asarray(W_v, dtype=np.float32)
    W_o = np.asarray(W_o, dtype=np.float32)
    if _NC_CACHE is None:
        _NC_CACHE = build()
    nc = _NC_CACHE

    in_maps = _build_in_maps(x, W_q, W_k, W_v, W_o)
    res = run_bass_kernel_spmd(nc, in_maps, core_ids=list(range(8)))
    Y = np.empty((2, S, D), dtype=np.float32)
    for c in range(8):
        g, r = divmod(c, 4)
        o = res.results[c]["out"]
        for s_idx in range(4):
            t = 4 * s_idx + r
            Y[g, t * 128 : (t + 1) * 128, :] = o[s_idx * 128 : (s_idx + 1) * 128, :]
    return Y
